# revision 22
# baseline (speedup 1.0000x reference)
"""AFNO2D Trainium2 kernel — 8 NeuronCores, no collectives.

Sharding: core = (b, c_half): b = core // 2, channels c_half*384 .. +384
(4 independent MLP blocks of 96 channels per core). Every stage is local.

The 2D Hartley transform (Re(FFT2) - Im(FFT2)) and its inverse are computed
as dense matmuls against precomputed cos/sin matrices (H=90, W=180 fixed):
  forward:  Xp = PW @ x (over w), Xm = MW @ x;  Xk = RC @ Xp + RS @ Xm (over h)
  block-MLP per spectral point (2 layers, relu, softshrink)
  inverse:  T = RC @ s, U = RS @ s (over k1);  y = CWI @ [T;U] (over k2) + x

Layout hops (matmul contraction must sit on SBUF partitions):
  h<->k2 swaps via strided SBUF->SBUF DMA (c-contiguous runs)
  c<->k1 swaps via TensorE transposes (96-wide tiles)
"""

import sys
import numpy as np

sys.path.insert(0, "/opt/trn_rl_repo")

import ml_dtypes  # noqa: E402
import concourse.bass as bass  # noqa: E402
import concourse.mybir as mybir  # noqa: E402
import concourse.tile as tile  # noqa: E402
from concourse import bacc  # noqa: E402
from concourse.bass_utils import run_bass_kernel_spmd  # noqa: E402
from concourse.masks import make_identity  # noqa: E402

# problem constants (hardcoded per spec)
B, H, W, C = 4, 90, 180, 768
K2 = 46                 # kept width modes (W//2+1 -> int((H//2+1)*1.0) slice)
NB, BLK = 8, 96
LAM = 0.01
CPC = 384               # channels per core
NBPC = 4                # MLP blocks per core
NPASS = 4               # c-passes of 96 channels (1 block) per core
WCH = 90                # w-chunk (contract dim per matmul), 2 chunks
HCH = 5                 # h-chunk for N=480 matmul free dims
K2CH = 5                # k2-chunk -> N=480
TGRP = 8                # transposes batched per PSUM bank

F32 = mybir.dt.float32
F32R = mybir.dt.float32r
BF16 = mybir.dt.bfloat16
BF = ml_dtypes.bfloat16


def _make_host_mats():
    w = np.arange(W)
    h = np.arange(H)
    k2 = np.arange(K2)
    beta = 2 * np.pi * np.outer(k2, w) / W            # [K2, W]
    PW = (np.cos(beta) + np.sin(beta)) / np.sqrt(W)
    MW = (np.cos(beta) - np.sin(beta)) / np.sqrt(W)
    alpha = 2 * np.pi * np.outer(h, h) / H            # [H, H] symmetric
    RC = (np.cos(alpha) / np.sqrt(H))                 # cos row mat (fwd + inv)
    RS = (np.sin(alpha) / np.sqrt(H))                 # sin row mat
    # MM1 lhsT per w-chunk: wf[j][w', m] : m<46 -> PW, m>=46 -> MW
    wf = np.zeros((2, WCH, 2 * K2), np.float32)
    for j in range(2):
        wf[j, :, :K2] = PW[:, j * WCH:(j + 1) * WCH].T
        wf[j, :, K2:] = MW[:, j * WCH:(j + 1) * WCH].T
    # WI lhsT per w-chunk: cwi[j][m, w'] : rows m<46 = cos(2pi w m/W)/sqrt(W),
    # m>=46 = -sin(...)
    cwi = np.zeros((2, 2 * K2, WCH), np.float32)
    for j in range(2):
        ww = w[j * WCH:(j + 1) * WCH]
        bb = 2 * np.pi * np.outer(k2, ww) / W
        cwi[j, :K2] = np.cos(bb) / np.sqrt(W)
        cwi[j, K2:] = -np.sin(bb) / np.sqrt(W)
    return (wf.astype(BF), RC.astype(BF), RS.astype(BF),
            cwi.astype(BF))


def _build_graph():
    nc = bacc.Bacc("TRN2", target_bir_lowering=False, debug=False, num_devices=8)

    x_ext = nc.declare_dram_parameter("x", [H, W, CPC], F32, isOutput=False)
    out_ext = nc.declare_dram_parameter("out", [H, W, CPC], F32, isOutput=True)
    wf_ext = nc.declare_dram_parameter("wf", [2, WCH, 2 * K2], BF16, isOutput=False)
    rc_ext = nc.declare_dram_parameter("rc", [H, H], BF16, isOutput=False)
    rs_ext = nc.declare_dram_parameter("rs", [H, H], BF16, isOutput=False)
    cwi_ext = nc.declare_dram_parameter("cwi", [2, 2 * K2, WCH], BF16, isOutput=False)
    w1_ext = nc.declare_dram_parameter("w1b", [NBPC, BLK, BLK], BF16, isOutput=False)
    w2_ext = nc.declare_dram_parameter("w2b", [NBPC, BLK, BLK], BF16, isOutput=False)
    b1_ext = nc.declare_dram_parameter("b1b", [BLK, NBPC], F32, isOutput=False)
    # b2 folded with softshrink: b2m = b2 - lam (relu path 1), b2p = -b2 - lam
    b2m_ext = nc.declare_dram_parameter("b2m", [BLK, NBPC], F32, isOutput=False)
    b2p_ext = nc.declare_dram_parameter("b2p", [BLK, NBPC], F32, isOutput=False)

    with tile.TileContext(nc) as tc:
        with (
            tc.tile_pool(name="consts", bufs=1) as consts,
            tc.tile_pool(name="xt", bufs=2) as xt_pool,
            tc.tile_pool(name="s1", bufs=1) as s1_pool,
            tc.tile_pool(name="s1t", bufs=1) as s1t_pool,
            tc.tile_pool(name="spec", bufs=1) as spec_pool,
            tc.tile_pool(name="ttut", bufs=1) as ttut_pool,
            tc.tile_pool(name="tu", bufs=1) as tu_pool,
            tc.tile_pool(name="ytile", bufs=4) as y_pool,
            tc.tile_pool(name="shr", bufs=3) as shr_pool,
            tc.tile_pool(name="mmps", bufs=2, space="PSUM") as mm_psum,
            tc.tile_pool(name="trps", bufs=2, space="PSUM") as tr_psum,
        ):
            # ---- constants to SBUF ----
            wf_sb = consts.tile([WCH, 2, 2 * K2], BF16)
            nc.sync.dma_start(wf_sb[:], wf_ext[:].rearrange("j w m -> w j m"))
            rc_sb = consts.tile([H, H], BF16)
            nc.sync.dma_start(rc_sb[:], rc_ext[:])
            rs_sb = consts.tile([H, H], BF16)
            nc.sync.dma_start(rs_sb[:], rs_ext[:])
            cwi_sb = consts.tile([2 * K2, 2, WCH], BF16)
            nc.sync.dma_start(cwi_sb[:], cwi_ext[:].rearrange("j m w -> m j w"))
            w1_sb = consts.tile([BLK, NBPC, BLK], BF16)
            nc.sync.dma_start(w1_sb[:], w1_ext[:].rearrange("n i o -> i n o"))
            w2_sb = consts.tile([BLK, NBPC, BLK], BF16)
            nc.sync.dma_start(w2_sb[:], w2_ext[:].rearrange("n i o -> i n o"))
            b1_sb = consts.tile([BLK, NBPC], F32)
            nc.sync.dma_start(b1_sb[:], b1_ext[:])
            b2m_sb = consts.tile([BLK, NBPC], F32)
            nc.sync.dma_start(b2m_sb[:], b2m_ext[:])
            b2p_sb = consts.tile([BLK, NBPC], F32)
            nc.sync.dma_start(b2p_sb[:], b2p_ext[:])
            ident = consts.tile([128, 128], BF16)
            make_identity(nc, ident[:])

            for p in range(NPASS):
                cs = p * BLK

                # ---- load x slice in w-partition layout ----
                xt = []
                for j in range(2):
                    t = xt_pool.tile([WCH, H, BLK], BF16, tag="xt")
                    nc.gpsimd.dma_start(
                        t[:],
                        x_ext[:, j * WCH:(j + 1) * WCH, cs:cs + BLK]
                        .rearrange("h w c -> w h c"),
                    )
                    xt.append(t)

                # ---- MM1: width-forward (contract w), out [2K2, h, c] ----
                s1 = s1_pool.tile([2 * K2, H, BLK], BF16)
                for hi in range(H // HCH):
                    hs = slice(hi * HCH, (hi + 1) * HCH)
                    ps = mm_psum.tile([2 * K2, HCH * BLK], F32, tag="mm")
                    nc.tensor.matmul(
                        ps[:],
                        lhsT=wf_sb[:, 0, :],
                        rhs=xt[0][:, hs, :].rearrange("w h c -> w (h c)"),
                        start=True, stop=False,
                    )
                    nc.tensor.matmul(
                        ps[:],
                        lhsT=wf_sb[:, 1, :],
                        rhs=xt[1][:, hs, :].rearrange("w h c -> w (h c)"),
                        start=False, stop=True,
                    )
                    nc.scalar.copy(
                        s1[:, hs, :].rearrange("m h c -> m (h c)"), ps[:]
                    )

                # ---- T1: TensorE transposes [k2pair, h] -> [h, k2pair] per c ----
                s1t = s1t_pool.tile([H, 2 * K2, BLK], BF16)
                for cg in range(0, BLK, TGRP):
                    cn = min(TGRP, BLK - cg)
                    pst = tr_psum.tile([H, TGRP, 2 * K2], BF16, tag="tr")
                    for c in range(cn):
                        nc.tensor.transpose(
                            pst[:, c, :], s1[:, :, cg + c], ident[:2 * K2, :2 * K2]
                        )
                    nc.vector.tensor_copy(
                        s1t[:, :, bass.ds(cg, cn)].rearrange("h m c -> h c m"),
                        pst[:, :cn, :],
                    )

                # ---- MM2: row-forward (contract h), out Xk [k1, k2, c] ----
                xk = spec_pool.tile([H, K2, BLK], BF16, tag="xk")
                nk2 = (K2 + K2CH - 1) // K2CH
                for ki in range(nk2):
                    k0 = ki * K2CH
                    kn = min(K2CH, K2 - k0)
                    ks = slice(k0, k0 + kn)
                    ps = mm_psum.tile([H, HCH * BLK], F32, tag="mm")
                    pss = ps[:, :kn * BLK]
                    nc.tensor.matmul(
                        pss, lhsT=rc_sb[:],
                        rhs=s1t[:, ks, :].rearrange("h m c -> h (m c)"),
                        start=True, stop=False,
                    )
                    nc.tensor.matmul(
                        pss, lhsT=rs_sb[:],
                        rhs=s1t[:, bass.ds(K2 + k0, kn), :]
                        .rearrange("h m c -> h (m c)"),
                        start=False, stop=True,
                    )
                    nc.vector.tensor_copy(
                        xk[:, ks, :].rearrange("k m c -> k (m c)"), pss
                    )

                # ---- T2: TensorE transposes [k1,c] -> [c,k1] per k2 ----
                xkt = spec_pool.tile([BLK, K2, H], BF16, tag="xkt")
                for kg in range(0, K2, TGRP):
                    kn = min(TGRP, K2 - kg)
                    pst = tr_psum.tile([BLK, TGRP, H], BF16, tag="tr")
                    for k in range(kn):
                        nc.tensor.transpose(
                            pst[:, k, :], xk[:, kg + k, :], ident[:H, :H]
                        )
                    nc.vector.tensor_copy(
                        xkt[:, bass.ds(kg, kn), :].rearrange("c m k -> c (m k)"),
                        pst[:, :kn, :].rearrange("c m k -> c (m k)"),
                    )

                # ---- MLP layer 1 (contract c-block) + bias + relu ----
                o1 = spec_pool.tile([BLK, K2, H], BF16, tag="o1")
                for ki in range(nk2):
                    k0 = ki * K2CH
                    kn = min(K2CH, K2 - k0)
                    ps = mm_psum.tile([BLK, K2CH * H], F32, tag="mlp")
                    pss = ps[:, :kn * H]
                    nc.tensor.matmul(
                        pss, lhsT=w1_sb[:, p, :],
                        rhs=xkt[:, bass.ds(k0, kn), :].rearrange("c m k -> c (m k)"),
                        start=True, stop=True,
                    )
                    nc.scalar.activation(
                        o1[:, bass.ds(k0, kn), :].rearrange("c m k -> c (m k)"),
                        pss, mybir.ActivationFunctionType.Relu,
                        bias=b1_sb[:, p:p + 1],
                    )

                # ---- MLP layer 2 + bias + softshrink ----
                o2s = spec_pool.tile([BLK, K2, H], BF16, tag="o2s")
                for ki in range(nk2):
                    k0 = ki * K2CH
                    kn = min(K2CH, K2 - k0)
                    ps = mm_psum.tile([BLK, K2CH * H], F32, tag="mlp")
                    pss = ps[:, :kn * H]
                    nc.tensor.matmul(
                        pss, lhsT=w2_sb[:, p, :],
                        rhs=o1[:, bass.ds(k0, kn), :].rearrange("c m k -> c (m k)"),
                        start=True, stop=True,
                    )
                    # softshrink(v + b2) = relu(v + b2 - lam) - relu(-v - b2 - lam)
                    sp = shr_pool.tile([BLK, K2CH * H], BF16, tag="shr_p")
                    sn = shr_pool.tile([BLK, K2CH * H], BF16, tag="shr_n")
                    nc.scalar.activation(
                        sp[:, :kn * H], pss, mybir.ActivationFunctionType.Relu,
                        bias=b2m_sb[:, p:p + 1], scale=1.0,
                    )
                    nc.scalar.activation(
                        sn[:, :kn * H], pss, mybir.ActivationFunctionType.Relu,
                        bias=b2p_sb[:, p:p + 1], scale=-1.0,
                    )
                    nc.vector.tensor_sub(
                        o2s[:, bass.ds(k0, kn), :].rearrange("c m k -> c (m k)"),
                        sp[:, :kn * H], sn[:, :kn * H],
                    )

                # ---- T3: TensorE transposes [c,k1] -> [k1,c] per k2 ----
                s3 = spec_pool.tile([H, K2, BLK], BF16, tag="s3")
                for kg in range(0, K2, TGRP):
                    kn = min(TGRP, K2 - kg)
                    pst = tr_psum.tile([H, TGRP, BLK], BF16, tag="tr3")
                    for k in range(kn):
                        nc.tensor.transpose(
                            pst[:, k, :], o2s[:, kg + k, :], ident[:BLK, :BLK]
                        )
                    nc.vector.tensor_copy(
                        s3[:, bass.ds(kg, kn), :].rearrange("k m c -> k (m c)"),
                        pst[:, :kn, :].rearrange("k m c -> k (m c)"),
                    )

                # ---- RI: row-inverse (contract k1): T = RC@s, U = RS@s ----
                # T lands at m<46, U at m>=46 of one [h, k2pair, c] tile.
                ttut = ttut_pool.tile([H, 2 * K2, BLK], BF16)
                for ki in range(nk2):
                    k0 = ki * K2CH
                    kn = min(K2CH, K2 - k0)
                    rhs = s3[:, bass.ds(k0, kn), :].rearrange("k m c -> k (m c)")
                    psT = mm_psum.tile([H, K2CH * BLK], F32, tag="mm")
                    nc.tensor.matmul(psT[:, :kn * BLK], lhsT=rc_sb[:], rhs=rhs,
                                     start=True, stop=True)
                    nc.scalar.activation(
                        ttut[:, bass.ds(k0, kn), :].rearrange("k m c -> k (m c)"),
                        psT[:, :kn * BLK],
                        mybir.ActivationFunctionType.Copy,
                    )
                    psU = mm_psum.tile([H, K2CH * BLK], F32, tag="mm")
                    nc.tensor.matmul(psU[:, :kn * BLK], lhsT=rs_sb[:], rhs=rhs,
                                     start=True, stop=True)
                    nc.vector.tensor_copy(
                        ttut[:, bass.ds(K2 + k0, kn), :].rearrange("k m c -> k (m c)"),
                        psU[:, :kn * BLK],
                    )

                # ---- T4: TensorE transposes [h, k2pair] -> [k2pair, h] per c ----
                tu = tu_pool.tile([2 * K2, H, BLK], BF16)
                for cg in range(0, BLK, TGRP):
                    cn = min(TGRP, BLK - cg)
                    pst = tr_psum.tile([2 * K2, TGRP, H], BF16, tag="tr")
                    for c in range(cn):
                        nc.tensor.transpose(
                            pst[:, c, :], ttut[:, :, cg + c], ident[:H, :H]
                        )
                    nc.vector.tensor_copy(
                        tu[:, :, bass.ds(cg, cn)].rearrange("m h c -> m c h"),
                        pst[:, :cn, :],
                    )

                # ---- WI: width-inverse (contract k2 pair) + residual + store ----
                for j in range(2):
                    for hi in range(H // HCH):
                        hs = slice(hi * HCH, (hi + 1) * HCH)
                        ps = mm_psum.tile([WCH, HCH * BLK], F32, tag="mm")
                        nc.tensor.matmul(
                            ps[:],
                            lhsT=cwi_sb[:, j, :],
                            rhs=tu[:, hs, :].rearrange("m h c -> m (h c)"),
                            start=True, stop=True,
                        )
                        ysb = y_pool.tile([WCH, HCH, BLK], F32, tag="y")
                        nc.vector.tensor_add(
                            ysb[:].rearrange("w h c -> w (h c)"), ps[:],
                            xt[j][:, hs, :].rearrange("w h c -> w (h c)"),
                        )
                        nc.sync.dma_start(
                            out_ext[hs, j * WCH:(j + 1) * WCH, cs:cs + BLK]
                            .rearrange("h w c -> w h c"),
                            ysb[:],
                        )

    nc.compile()
    return nc


_CACHE = {}


def _get_graph():
    if "nc" not in _CACHE:
        _CACHE["nc"] = _build_graph()
    return _CACHE["nc"]


def kernel(x, w1, b1, w2, b2):
    x = np.ascontiguousarray(np.asarray(x, dtype=np.float32))
    w1 = np.asarray(w1, dtype=np.float32)
    b1 = np.asarray(b1, dtype=np.float32)
    w2 = np.asarray(w2, dtype=np.float32)
    b2 = np.asarray(b2, dtype=np.float32)

    wf, rc, rs, cwi = _make_host_mats()
    nc = _get_graph()

    in_maps = []
    for core in range(8):
        b = core // 2
        half = core % 2
        cs = half * CPC
        nb0 = half * NBPC
        b2c = b2[0, nb0:nb0 + NBPC]        # [NBPC, BLK]
        in_maps.append({
            "x": np.ascontiguousarray(x[b, :, :, cs:cs + CPC]),
            "wf": wf,
            "rc": rc,
            "rs": rs,
            "cwi": cwi,
            "w1b": w1[0, nb0:nb0 + NBPC].astype(BF),
            "w2b": w2[0, nb0:nb0 + NBPC].astype(BF),
            "b1b": np.ascontiguousarray(b1[0, nb0:nb0 + NBPC].T.astype(np.float32)),
            "b2m": np.ascontiguousarray((b2c - LAM).T.astype(np.float32)),
            "b2p": np.ascontiguousarray((-b2c - LAM).T.astype(np.float32)),
        })

    res = run_bass_kernel_spmd(nc, in_maps, core_ids=list(range(8)),
                               **_CACHE.get("run_kwargs", {}))
    _CACHE["last_result"] = res

    y = np.empty((B, H, W, C), np.float32)
    for core in range(8):
        b = core // 2
        cs = (core % 2) * CPC
        y[b, :, :, cs:cs + CPC] = res.results[core]["out"]
    return y


if __name__ == "__main__":
    xs = np.random.randn(B, H, W, C).astype(np.float32)
    w1s = 0.02 * np.random.randn(2, NB, BLK, BLK).astype(np.float32)
    b1s = 0.02 * np.random.randn(2, NB, BLK).astype(np.float32)
    w2s = 0.02 * np.random.randn(2, NB, BLK, BLK).astype(np.float32)
    b2s = 0.02 * np.random.randn(2, NB, BLK).astype(np.float32)
    out = kernel(x=xs, w1=w1s, b1=b1s, w2=w2s, b2=b2s)
    print("ran, out shape", out.shape)


# revision 31
# speedup vs baseline: 1.5936x; 1.5936x over previous
"""AFNO2D Trainium2 kernel — 8 NeuronCores, no collectives.

Sharding: core = (b, c_half): b = core // 2, channels c_half*384 .. +384
(4 independent MLP blocks of 96 channels per core). Every stage is local.

The 2D Hartley transform (Re(FFT2) - Im(FFT2)) and its inverse are computed
as dense matmuls against precomputed cos/sin matrices (H=90, W=180 fixed):
  forward:  Xp = PW @ x (over w), Xm = MW @ x;  Xk = RC @ Xp + RS @ Xm (over h)
  block-MLP per spectral point (2 layers, relu, softshrink)
  inverse:  T = RC @ s, U = RS @ s (over k1);  y = CWI @ [T;U] (over k2) + x

Layout hops (matmul contraction must sit on SBUF partitions):
  h<->k2 swaps via strided SBUF->SBUF DMA (c-contiguous runs)
  c<->k1 swaps via TensorE transposes (96-wide tiles)
"""

import sys
import numpy as np

sys.path.insert(0, "/opt/trn_rl_repo")

import ml_dtypes  # noqa: E402
import concourse.bass as bass  # noqa: E402
import concourse.mybir as mybir  # noqa: E402
import concourse.tile as tile  # noqa: E402
from concourse import bacc  # noqa: E402
from concourse.bass_utils import run_bass_kernel_spmd  # noqa: E402
from concourse.masks import make_identity  # noqa: E402

# problem constants (hardcoded per spec)
B, H, W, C = 4, 90, 180, 768
K2 = 46                 # kept width modes (W//2+1 -> int((H//2+1)*1.0) slice)
NB, BLK = 8, 96
LAM = 0.01
CPC = 384               # channels per core
NBPC = 4                # MLP blocks per core
NPASS = 4               # c-passes of 96 channels (1 block) per core
WCH = 90                # w-chunk (contract dim per matmul), 2 chunks
HCH = 5                 # h-chunk for N=480 matmul free dims
K2CH = 5                # k2-chunk -> N=480
TGRP = 8                # transposes batched per PSUM bank
CCH = 10                # c-chunk for MM2 (N = 10*46 = 460)
YGRP = 3                # WI h-chunks batched per store DMA

F32 = mybir.dt.float32
F32R = mybir.dt.float32r
BF16 = mybir.dt.bfloat16
BF = ml_dtypes.bfloat16


def _make_host_mats():
    w = np.arange(W)
    h = np.arange(H)
    k2 = np.arange(K2)
    beta = 2 * np.pi * np.outer(k2, w) / W            # [K2, W]
    PW = (np.cos(beta) + np.sin(beta)) / np.sqrt(W)
    MW = (np.cos(beta) - np.sin(beta)) / np.sqrt(W)
    alpha = 2 * np.pi * np.outer(h, h) / H            # [H, H] symmetric
    RC = (np.cos(alpha) / np.sqrt(H))                 # cos row mat (fwd + inv)
    RS = (np.sin(alpha) / np.sqrt(H))                 # sin row mat
    # MM1 lhsT per w-chunk: wf[j][w', m] : m<46 -> PW, m>=46 -> MW
    wf = np.zeros((2, WCH, 2 * K2), np.float32)
    for j in range(2):
        wf[j, :, :K2] = PW[:, j * WCH:(j + 1) * WCH].T
        wf[j, :, K2:] = MW[:, j * WCH:(j + 1) * WCH].T
    # WI lhsT per w-chunk: cwi[j][m, w'] : rows m<46 = cos(2pi w m/W)/sqrt(W),
    # m>=46 = -sin(...)
    cwi = np.zeros((2, 2 * K2, WCH), np.float32)
    for j in range(2):
        ww = w[j * WCH:(j + 1) * WCH]
        bb = 2 * np.pi * np.outer(k2, ww) / W
        cwi[j, :K2] = np.cos(bb) / np.sqrt(W)
        cwi[j, K2:] = -np.sin(bb) / np.sqrt(W)
    return (wf.astype(BF), RC.astype(BF), RS.astype(BF),
            cwi.astype(BF))


def _build_graph():
    nc = bacc.Bacc("TRN2", target_bir_lowering=False, debug=False, num_devices=8)

    x_ext = nc.declare_dram_parameter("x", [H, W, CPC], F32, isOutput=False)
    out_ext = nc.declare_dram_parameter("out", [H, W, CPC], F32, isOutput=True)
    wf_ext = nc.declare_dram_parameter("wf", [2, WCH, 2 * K2], BF16, isOutput=False)
    rc_ext = nc.declare_dram_parameter("rc", [H, H], BF16, isOutput=False)
    rs_ext = nc.declare_dram_parameter("rs", [H, H], BF16, isOutput=False)
    cwi_ext = nc.declare_dram_parameter("cwi", [2, 2 * K2, WCH], BF16, isOutput=False)
    w1_ext = nc.declare_dram_parameter("w1b", [NBPC, BLK, BLK], BF16, isOutput=False)
    w2_ext = nc.declare_dram_parameter("w2b", [NBPC, BLK, BLK], BF16, isOutput=False)
    b1_ext = nc.declare_dram_parameter("b1b", [BLK, NBPC], F32, isOutput=False)
    # b2 folded with softshrink: b2m = b2 - lam (relu path 1), b2p = -b2 - lam
    b2m_ext = nc.declare_dram_parameter("b2m", [BLK, NBPC], F32, isOutput=False)
    b2p_ext = nc.declare_dram_parameter("b2p", [BLK, NBPC], F32, isOutput=False)

    with tile.TileContext(nc) as tc:
        with (
            tc.tile_pool(name="consts", bufs=1) as consts,
            tc.tile_pool(name="xt", bufs=2) as xt_pool,
            tc.tile_pool(name="s1", bufs=1) as s1_pool,
            tc.tile_pool(name="s1t", bufs=1) as s1t_pool,
            tc.tile_pool(name="spec", bufs=1) as spec_pool,
            tc.tile_pool(name="ttut", bufs=1) as ttut_pool,
            tc.tile_pool(name="tu", bufs=1) as tu_pool,
            tc.tile_pool(name="ytile", bufs=2) as y_pool,
            tc.tile_pool(name="shr", bufs=3) as shr_pool,
            tc.tile_pool(name="mmps", bufs=3, space="PSUM") as mm_psum,
            tc.tile_pool(name="trps", bufs=3, space="PSUM") as tr_psum,
        ):
            # ---- constants to SBUF ----
            wf_sb = consts.tile([WCH, 2, 2 * K2], BF16)
            nc.sync.dma_start(wf_sb[:], wf_ext[:].rearrange("j w m -> w j m"))
            rc_sb = consts.tile([H, H], BF16)
            nc.sync.dma_start(rc_sb[:], rc_ext[:])
            rs_sb = consts.tile([H, H], BF16)
            nc.sync.dma_start(rs_sb[:], rs_ext[:])
            cwi_sb = consts.tile([2 * K2, 2, WCH], BF16)
            nc.sync.dma_start(cwi_sb[:], cwi_ext[:].rearrange("j m w -> m j w"))
            w1_sb = consts.tile([BLK, NBPC, BLK], BF16)
            nc.sync.dma_start(w1_sb[:], w1_ext[:].rearrange("n i o -> i n o"))
            w2_sb = consts.tile([BLK, NBPC, BLK], BF16)
            nc.sync.dma_start(w2_sb[:], w2_ext[:].rearrange("n i o -> i n o"))
            b1_sb = consts.tile([BLK, NBPC], F32)
            nc.sync.dma_start(b1_sb[:], b1_ext[:])
            b2m_sb = consts.tile([BLK, NBPC], F32)
            nc.sync.dma_start(b2m_sb[:], b2m_ext[:])
            b2p_sb = consts.tile([BLK, NBPC], F32)
            nc.sync.dma_start(b2p_sb[:], b2p_ext[:])
            ident = consts.tile([128, 128], BF16)
            make_identity(nc, ident[:])

            for p in range(NPASS):
                cs = p * BLK

                # ---- load x slice in w-partition layout ----
                xt = []
                for j in range(2):
                    t = xt_pool.tile([WCH, H, BLK], BF16, tag="xt")
                    nc.gpsimd.dma_start(
                        t[:],
                        x_ext[:, j * WCH:(j + 1) * WCH, cs:cs + BLK]
                        .rearrange("h w c -> w h c"),
                    )
                    xt.append(t)

                # ---- MM1: width-forward (contract w), out [2K2, h, c] ----
                s1 = s1_pool.tile([2 * K2, H, BLK], BF16)
                for hi in range(H // HCH):
                    hs = slice(hi * HCH, (hi + 1) * HCH)
                    ps = mm_psum.tile([2 * K2, HCH * BLK], F32, tag="mm")
                    nc.tensor.matmul(
                        ps[:],
                        lhsT=wf_sb[:, 0, :],
                        rhs=xt[0][:, hs, :].rearrange("w h c -> w (h c)"),
                        start=True, stop=False,
                    )
                    nc.tensor.matmul(
                        ps[:],
                        lhsT=wf_sb[:, 1, :],
                        rhs=xt[1][:, hs, :].rearrange("w h c -> w (h c)"),
                        start=False, stop=True,
                    )
                    nc.scalar.copy(
                        s1[:, hs, :].rearrange("m h c -> m (h c)"), ps[:]
                    )

                # ---- T1: TensorE transposes [k2pair, h] -> [h, k2pair] per c ----
                # s1t layout [h, c, m] so the evacuation writes are dense.
                s1t = s1t_pool.tile([H, BLK, 2 * K2], BF16)
                for cg in range(0, BLK, TGRP):
                    cn = min(TGRP, BLK - cg)
                    pst = tr_psum.tile([H, TGRP, 2 * K2], BF16, tag="tr")
                    for c in range(cn):
                        nc.tensor.transpose(
                            pst[:, c, :], s1[:, :, cg + c], ident[:2 * K2, :2 * K2]
                        )
                    nc.vector.tensor_copy(
                        s1t[:, bass.ds(cg, cn), :].rearrange("h c m -> h (c m)"),
                        pst[:, :cn, :].rearrange("h c m -> h (c m)"),
                    )

                # ---- MM2: row-forward (contract h), out Xk [k1, c, k2] ----
                xk = spec_pool.tile([H, BLK, K2], BF16, tag="xk")
                nk2 = (K2 + K2CH - 1) // K2CH
                ncch = (BLK + CCH - 1) // CCH
                for ci in range(ncch):
                    c0 = ci * CCH
                    cn = min(CCH, BLK - c0)
                    csl = bass.ds(c0, cn)
                    ps = mm_psum.tile([H, CCH * K2], F32, tag="mm")
                    pss = ps[:, :cn * K2]
                    nc.tensor.matmul(
                        pss, lhsT=rc_sb[:], rhs=s1t[:, csl, :K2],
                        start=True, stop=False,
                    )
                    nc.tensor.matmul(
                        pss, lhsT=rs_sb[:], rhs=s1t[:, csl, K2:],
                        start=False, stop=True,
                    )
                    nc.vector.tensor_copy(
                        xk[:, csl, :].rearrange("k c m -> k (c m)"), pss
                    )

                # ---- T2: TensorE transposes [k1,c] -> [c,k1] per k2 ----
                xkt = spec_pool.tile([BLK, K2, H], BF16, tag="xkt")
                for kg in range(0, K2, TGRP):
                    kn = min(TGRP, K2 - kg)
                    pst = tr_psum.tile([BLK, TGRP, H], BF16, tag="tr")
                    for k in range(kn):
                        nc.tensor.transpose(
                            pst[:, k, :], xk[:, :, kg + k], ident[:H, :H]
                        )
                    nc.vector.tensor_copy(
                        xkt[:, bass.ds(kg, kn), :].rearrange("c m k -> c (m k)"),
                        pst[:, :kn, :].rearrange("c m k -> c (m k)"),
                    )

                # ---- MLP layer 1 (contract c-block) + bias + relu ----
                o1 = spec_pool.tile([BLK, K2, H], BF16, tag="o1")
                for ki in range(nk2):
                    k0 = ki * K2CH
                    kn = min(K2CH, K2 - k0)
                    ps = mm_psum.tile([BLK, K2CH * H], F32, tag="mlp", bufs=2)
                    pss = ps[:, :kn * H]
                    nc.tensor.matmul(
                        pss, lhsT=w1_sb[:, p, :],
                        rhs=xkt[:, bass.ds(k0, kn), :].rearrange("c m k -> c (m k)"),
                        start=True, stop=True,
                    )
                    nc.scalar.activation(
                        o1[:, bass.ds(k0, kn), :].rearrange("c m k -> c (m k)"),
                        pss, mybir.ActivationFunctionType.Relu,
                        bias=b1_sb[:, p:p + 1],
                    )

                # ---- MLP layer 2 + bias + softshrink ----
                o2s = spec_pool.tile([BLK, K2, H], BF16, tag="o2s")
                for ki in range(nk2):
                    k0 = ki * K2CH
                    kn = min(K2CH, K2 - k0)
                    ps = mm_psum.tile([BLK, K2CH * H], F32, tag="mlp", bufs=2)
                    pss = ps[:, :kn * H]
                    nc.tensor.matmul(
                        pss, lhsT=w2_sb[:, p, :],
                        rhs=o1[:, bass.ds(k0, kn), :].rearrange("c m k -> c (m k)"),
                        start=True, stop=True,
                    )
                    # softshrink(v + b2) = relu(v + b2 - lam) - relu(-v - b2 - lam)
                    sp = shr_pool.tile([BLK, K2CH * H], BF16, tag="shr_p")
                    sn = shr_pool.tile([BLK, K2CH * H], BF16, tag="shr_n")
                    nc.scalar.activation(
                        sp[:, :kn * H], pss, mybir.ActivationFunctionType.Relu,
                        bias=b2m_sb[:, p:p + 1], scale=1.0,
                    )
                    nc.scalar.activation(
                        sn[:, :kn * H], pss, mybir.ActivationFunctionType.Relu,
                        bias=b2p_sb[:, p:p + 1], scale=-1.0,
                    )
                    nc.vector.tensor_sub(
                        o2s[:, bass.ds(k0, kn), :].rearrange("c m k -> c (m k)"),
                        sp[:, :kn * H], sn[:, :kn * H],
                    )

                # ---- T3: TensorE transposes [c,k1] -> [k1,c] per k2 ----
                s3 = spec_pool.tile([H, K2, BLK], BF16, tag="s3")
                for kg in range(0, K2, TGRP):
                    kn = min(TGRP, K2 - kg)
                    pst = tr_psum.tile([H, TGRP, BLK], BF16, tag="tr")
                    for k in range(kn):
                        nc.tensor.transpose(
                            pst[:, k, :], o2s[:, kg + k, :], ident[:BLK, :BLK]
                        )
                    nc.vector.tensor_copy(
                        s3[:, bass.ds(kg, kn), :].rearrange("k m c -> k (m c)"),
                        pst[:, :kn, :].rearrange("k m c -> k (m c)"),
                    )

                # ---- RI: row-inverse (contract k1): T = RC@s, U = RS@s ----
                # T lands at m<46, U at m>=46 of one [h, k2pair, c] tile.
                ttut = ttut_pool.tile([H, 2 * K2, BLK], BF16)
                for ki in range(nk2):
                    k0 = ki * K2CH
                    kn = min(K2CH, K2 - k0)
                    rhs = s3[:, bass.ds(k0, kn), :].rearrange("k m c -> k (m c)")
                    psT = mm_psum.tile([H, K2CH * BLK], F32, tag="mm")
                    nc.tensor.matmul(psT[:, :kn * BLK], lhsT=rc_sb[:], rhs=rhs,
                                     start=True, stop=True)
                    nc.scalar.activation(
                        ttut[:, bass.ds(k0, kn), :].rearrange("k m c -> k (m c)"),
                        psT[:, :kn * BLK],
                        mybir.ActivationFunctionType.Copy,
                    )
                    psU = mm_psum.tile([H, K2CH * BLK], F32, tag="mm")
                    nc.tensor.matmul(psU[:, :kn * BLK], lhsT=rs_sb[:], rhs=rhs,
                                     start=True, stop=True)
                    nc.vector.tensor_copy(
                        ttut[:, bass.ds(K2 + k0, kn), :].rearrange("k m c -> k (m c)"),
                        psU[:, :kn * BLK],
                    )

                # ---- T4: TensorE transposes [h, k2pair] -> [k2pair, h] per c ----
                # tu layout [m, c, h] so the evacuation writes are dense.
                tu = tu_pool.tile([2 * K2, BLK, H], BF16)
                for cg in range(0, BLK, TGRP):
                    cn = min(TGRP, BLK - cg)
                    pst = tr_psum.tile([2 * K2, TGRP, H], BF16, tag="tr")
                    for c in range(cn):
                        nc.tensor.transpose(
                            pst[:, c, :], ttut[:, :, cg + c], ident[:H, :H]
                        )
                    nc.vector.tensor_copy(
                        tu[:, bass.ds(cg, cn), :].rearrange("m c h -> m (c h)"),
                        pst[:, :cn, :].rearrange("m c h -> m (c h)"),
                    )

                # ---- WI: width-inverse (contract k2 pair) + residual + store ----
                # YGRP h-chunks accumulate into one SBUF tile per store DMA.
                for j in range(2):
                    for hg in range(H // (HCH * YGRP)):
                        h0g = hg * HCH * YGRP
                        ysb = y_pool.tile([WCH, HCH * YGRP, BLK], F32, tag="y")
                        for si in range(YGRP):
                            h0 = h0g + si * HCH
                            hs = slice(h0, h0 + HCH)
                            ps = mm_psum.tile([WCH, HCH * BLK], F32, tag="mm")
                            nc.tensor.matmul(
                                ps[:],
                                lhsT=cwi_sb[:, j, :],
                                rhs=tu[:, :, hs].rearrange("m c h -> m h c"),
                                start=True, stop=True,
                            )
                            nc.vector.tensor_add(
                                ysb[:, bass.ds(si * HCH, HCH), :]
                                .rearrange("w h c -> w (h c)"),
                                ps[:],
                                xt[j][:, hs, :].rearrange("w h c -> w (h c)"),
                            )
                        nc.sync.dma_start(
                            out_ext[h0g:h0g + HCH * YGRP,
                                    j * WCH:(j + 1) * WCH, cs:cs + BLK]
                            .rearrange("h w c -> w h c"),
                            ysb[:],
                        )

    nc.compile()
    return nc


_CACHE = {}


def _get_graph():
    if "nc" not in _CACHE:
        _CACHE["nc"] = _build_graph()
    return _CACHE["nc"]


def kernel(x, w1, b1, w2, b2):
    x = np.ascontiguousarray(np.asarray(x, dtype=np.float32))
    w1 = np.asarray(w1, dtype=np.float32)
    b1 = np.asarray(b1, dtype=np.float32)
    w2 = np.asarray(w2, dtype=np.float32)
    b2 = np.asarray(b2, dtype=np.float32)

    wf, rc, rs, cwi = _make_host_mats()
    nc = _get_graph()

    in_maps = []
    for core in range(8):
        b = core // 2
        half = core % 2
        cs = half * CPC
        nb0 = half * NBPC
        b2c = b2[0, nb0:nb0 + NBPC]        # [NBPC, BLK]
        in_maps.append({
            "x": np.ascontiguousarray(x[b, :, :, cs:cs + CPC]),
            "wf": wf,
            "rc": rc,
            "rs": rs,
            "cwi": cwi,
            "w1b": w1[0, nb0:nb0 + NBPC].astype(BF),
            "w2b": w2[0, nb0:nb0 + NBPC].astype(BF),
            "b1b": np.ascontiguousarray(b1[0, nb0:nb0 + NBPC].T.astype(np.float32)),
            "b2m": np.ascontiguousarray((b2c - LAM).T.astype(np.float32)),
            "b2p": np.ascontiguousarray((-b2c - LAM).T.astype(np.float32)),
        })

    res = run_bass_kernel_spmd(nc, in_maps, core_ids=list(range(8)),
                               **_CACHE.get("run_kwargs", {}))
    _CACHE["last_result"] = res

    y = np.empty((B, H, W, C), np.float32)
    for core in range(8):
        b = core // 2
        cs = (core % 2) * CPC
        y[b, :, :, cs:cs + CPC] = res.results[core]["out"]
    return y


if __name__ == "__main__":
    xs = np.random.randn(B, H, W, C).astype(np.float32)
    w1s = 0.02 * np.random.randn(2, NB, BLK, BLK).astype(np.float32)
    b1s = 0.02 * np.random.randn(2, NB, BLK).astype(np.float32)
    w2s = 0.02 * np.random.randn(2, NB, BLK, BLK).astype(np.float32)
    b2s = 0.02 * np.random.randn(2, NB, BLK).astype(np.float32)
    out = kernel(x=xs, w1=w1s, b1=b1s, w2=w2s, b2=b2s)
    print("ran, out shape", out.shape)
